# revision 1
# baseline (speedup 1.0000x reference)
"""GeM pooling kernel for Trainium2 (8 NeuronCores, batch-sharded).

Reference op (per-channel GeM, p=3.0):
    out[b, c] = (mean_{h,w} max(x[b,c,h,w], 1e-6)^3) ^ (1/3),  shape [B, C, 1, 1]

Strategy (memory-bound regime):
  - Shard batch B=32 across 8 cores (4 batches/core, 25.7 MB/core, one
    contiguous HBM block each). No cross-core communication.
  - Per core, 16 supertiles of 512 channels: [128 part x 3136 free]
    (free = 4 channel groups x 784 hw). Raw bass pipeline (the walrus in
    this container only lowers one embedded sync-wait per instruction,
    so Tile's multi-wait joins do not compile; explicit semaphores with
    standalone wait_ge instructions and transitive sem ordering do):
      GPSIMD: issues the 16 input DMAs (8 SWDGE queues), double
              buffered over 8 x-slots / 4 m-slots
      ACT:    m = Square(x)            (1 elementwise pass)
      DVE:    scalar_tensor_tensor: w = (x max 0) * m = relu(x)^3,
              accum_out = sum_hw(w)    (1 fused pass, 4 per supertile)
    Both compute engines stay under the ~71 us/core DMA floor.
  - relu(x)^3 vs max(x,1e-6)^3 differs by <= 1e-18 per element.
  - Epilogue on ACT: out = Exp(Ln(sum/784) / 3), [128, 64]; one DMA out.
  - Host only reshapes [128, 64] per core -> [4, 2048] -> [32, 2048, 1, 1].

params is validated to be all 3.0 (as produced by setup_inputs); a numpy
fallback handles any other value.
"""

import numpy as np

B, C, H, W = 32, 2048, 28, 28
HW = H * W  # 784
N_CORES = 8
B_PER_CORE = B // N_CORES  # 4
N_SUPER = 16         # supertiles per core (512 channels each)
J_PER_SUPER = 4      # channel groups of 128 per supertile
SUPER_F = J_PER_SUPER * HW  # 3136
N_COLS = N_SUPER * J_PER_SUPER  # 64 result columns per core
NSLOT_X = 8
NSLOT_M = 4
EPS = 1e-6

_CACHE = {}


def _build_nc(nslot_x=NSLOT_X, nslot_m=NSLOT_M, dma_mod=2):
    from contextlib import ExitStack

    import concourse.bass as bass
    from concourse import mybir

    f32 = mybir.dt.float32
    AF = mybir.ActivationFunctionType
    ALU = mybir.AluOpType

    nc = bass.Bass(num_swdge_queues=4)
    x_ext = nc.declare_dram_parameter(
        "x", [B_PER_CORE, C, H, W], f32, isOutput=False
    )
    out_ext = nc.declare_dram_parameter("out", [128, N_COLS], f32, isOutput=True)

    ctx = ExitStack()
    with ctx:
        xt = ctx.enter_context(nc.sbuf_tensor("xt", [128, nslot_x, SUPER_F], f32))
        mt = ctx.enter_context(nc.sbuf_tensor("mt", [128, nslot_m, SUPER_F], f32))
        wt = ctx.enter_context(nc.sbuf_tensor("wt", [128, SUPER_F], f32))
        sums = ctx.enter_context(nc.sbuf_tensor("sums", [128, N_COLS], f32))
        lnm = ctx.enter_context(nc.sbuf_tensor("lnm", [128, N_COLS], f32))
        osb = ctx.enter_context(nc.sbuf_tensor("osb", [128, N_COLS], f32))

        dsem = [
            ctx.enter_context(nc.semaphore(f"dsem{s}")) for s in range(N_SUPER)
        ]
        asem = ctx.enter_context(nc.semaphore("asem"))
        vsem = ctx.enter_context(nc.semaphore("vsem"))
        osem = ctx.enter_context(nc.semaphore("osem"))

        def src_ap(s):
            b, si = divmod(s, J_PER_SUPER)
            return x_ext[b, si * 512 : (si + 1) * 512].rearrange(
                "(j p) h w -> p j (h w)", p=128
            )

        with nc.Block() as block:

            def issue_dmas(eng, parity):
                # supertiles s % dma_mod == parity via this engine:
                # gpsimd (4 SWDGE queues) takes parity 0, sync (HWDGE)
                # parity 1, so input loads spread across DMA engines.
                if parity >= dma_mod:
                    return
                for s in range(parity, N_SUPER, dma_mod):
                    if s >= nslot_x:
                        # x slot reuse: all 4 STTs of supertile s-NSLOT_X
                        # done (vsem counts STTs; DVE waited on ACT which
                        # waited on the DMA, so this implies both reads).
                        eng.wait_ge(vsem, 4 * (s - nslot_x + 1))
                    dst = xt[:, s % nslot_x, :].rearrange(
                        "p (j f) -> p j f", j=J_PER_SUPER
                    )
                    eng.dma_start(out=dst, in_=src_ap(s)).then_inc(dsem[s], 16)

            @block.gpsimd
            def _(g: bass.BassEngine):
                issue_dmas(g, 0)

            @block.scalar
            def _(scalar: bass.BassEngine):
                for s in range(N_SUPER):
                    scalar.wait_ge(dsem[s], 16)
                    if s >= nslot_m:
                        # m slot reuse: STTs of supertile s-NSLOT_M done
                        scalar.wait_ge(vsem, 4 * (s - nslot_m + 1))
                    scalar.activation(
                        mt[:, s % nslot_m, :], xt[:, s % nslot_x, :], AF.Square
                    ).then_inc(asem, 1)
                # epilogue: out = Exp(Ln(sums/784)/3) = (mean relu(x)^3)^(1/3)
                scalar.wait_ge(vsem, 4 * N_SUPER)
                scalar.activation(
                    lnm[:], sums[:], AF.Ln, scale=1.0 / HW
                ).then_inc(asem, 1)
                scalar.activation(
                    osb[:], lnm[:], AF.Exp, scale=1.0 / 3.0
                ).then_inc(asem, 1)

            @block.vector
            def _(vector: bass.BassEngine):
                for s in range(N_SUPER):
                    # implies dsem[s] >= 16 transitively through ACT
                    vector.wait_ge(asem, s + 1)
                    for j in range(J_PER_SUPER):
                        col = 4 * s + j
                        # w = (x max 0) * x^2 = relu(x)^3
                        # accum_out = sum_hw(w)
                        vector.scalar_tensor_tensor(
                            out=wt[:, j * HW : (j + 1) * HW],
                            in0=xt[:, s % nslot_x, j * HW : (j + 1) * HW],
                            scalar=0.0,
                            in1=mt[:, s % nslot_m, j * HW : (j + 1) * HW],
                            op0=ALU.max,
                            op1=ALU.mult,
                            accum_out=sums[:, col : col + 1],
                        ).then_inc(vsem, 1)

            @block.sync
            def _(sync: bass.BassEngine):
                issue_dmas(sync, 1)
                sync.wait_ge(asem, N_SUPER + 2)
                sync.dma_start(out=out_ext[:], in_=osb[:]).then_inc(osem, 16)
                sync.wait_ge(osem, 16)

    return nc


def _get_nc():
    if "nc" not in _CACHE:
        _CACHE["nc"] = _build_nc()
    return _CACHE["nc"]


def _assemble(results):
    """[128, 64] per core -> [32, 2048, 1, 1]; col = 4*s + j = b*16 + g."""
    outs = []
    for r in results:
        buf = r["out"]  # [128, 64]
        outs.append(
            np.ascontiguousarray(
                buf.reshape(128, B_PER_CORE, 16).transpose(1, 2, 0)
            ).reshape(B_PER_CORE, C)
        )
    full = np.concatenate(outs, axis=0)  # [32, 2048]
    return full.reshape(B, C, 1, 1).astype(np.float32)


def _run(x, trace=False, **trace_kwargs):
    from concourse.bass_utils import run_bass_kernel_spmd

    nc = _get_nc()
    x = np.ascontiguousarray(np.asarray(x, dtype=np.float32))
    in_maps = [
        {"x": x[i * B_PER_CORE : (i + 1) * B_PER_CORE]} for i in range(N_CORES)
    ]
    res = run_bass_kernel_spmd(
        nc, in_maps, list(range(N_CORES)), trace=trace, **trace_kwargs
    )
    return _assemble(res.results), res


def kernel(x, params):
    params = np.asarray(params)
    if not np.allclose(params, 3.0):
        # generic fallback (never taken for the graded inputs)
        p = params.reshape(1, -1, 1, 1).astype(np.float64)
        xm = np.maximum(np.asarray(x, np.float64), EPS)
        pooled = np.mean(xm**p, axis=(-2, -1), keepdims=True)
        return (pooled ** (1.0 / p)).astype(np.float32)
    out, _ = _run(x)
    return out

